# revision 17
# baseline (speedup 1.0000x reference)
"""3-layer GCN on 8 Trainium2 NeuronCores (SPMD, Bass/Tile).

Strategy: shard destination nodes across cores via a balanced LPT partition
into 128-row blocks; replicate weights; per layer: local dense matmul (fp16)
-> AllGather of the fp16 support table into Shared DRAM -> SPMM as
per-128-edge-chunk [indirect-DMA row gather (fp16, 128B rows); one fused DVE
op building the weight-scaled one-hot selection matrix; PE matmul
accumulating the segment-sum in PSUM].  All per-core variation lives in the
data (index/weight arrays); the program is identical across cores.
"""
import heapq

import numpy as np

N_NODES = 50000
N_EDGES = 800000
NFEAT, NHID, NCLASS = 512, 64, 40
NCORES = 8
P = 128
BLOCKS = 49                 # blocks per core
NLOC = BLOCKS * P           # 6272 rows per core
NB = NCORES * BLOCKS        # 392 blocks total
NTOT = NCORES * NLOC        # 50176 padded nodes
SPLIT = 24                  # blocks in the first AllGather half

_cache = {}


def _partition_graph(row, col, edge_weight):
    """Host-side graph partitioning. Returns permutation, per-core packed
    edge arrays, and the uniform per-block chunk profile."""
    deg = np.bincount(row, minlength=N_NODES)
    order = np.argsort(-deg, kind="stable")

    # LPT: assign nodes (desc degree) to the min-edge-sum block with a free slot
    heap = [(0, b) for b in range(NB)]
    heapq.heapify(heap)
    counts = np.zeros(NB, np.int64)
    bsum = np.zeros(NB, np.int64)
    assign = np.empty(N_NODES, np.int64)
    within = np.empty(N_NODES, np.int64)
    for n in order:
        s, b = heapq.heappop(heap)
        d = int(deg[n])
        assign[n] = b
        within[n] = counts[b]
        counts[b] += 1
        bsum[b] += d
        if counts[b] < P:
            heapq.heappush(heap, (s + d, b))

    # blocks -> cores: snake deal by desc edge sum; within-core slot = round idx
    border = np.argsort(-bsum, kind="stable")
    core_of_block = np.empty(NB, np.int64)
    slot_of_block = np.empty(NB, np.int64)
    for i, b in enumerate(border):
        r, j = divmod(i, NCORES)
        core_of_block[b] = j if r % 2 == 0 else NCORES - 1 - j
        slot_of_block[b] = r

    # per-slot chunk profile (uniform across cores)
    slot_max = np.zeros(BLOCKS, np.int64)
    for b in range(NB):
        r = slot_of_block[b]
        slot_max[r] = max(slot_max[r], bsum[b])
    u_profile = np.maximum(1, np.ceil(slot_max / P).astype(np.int64))
    cstart = np.concatenate([[0], np.cumsum(u_profile)[:-1]]).astype(np.int64)
    nch = int(u_profile.sum())

    # node permutation old -> new
    perm = (core_of_block[assign] * NLOC + slot_of_block[assign] * P + within)

    g_r = perm[row]
    g_c = perm[col]
    core_e = g_r // NLOC
    bpos_e = (g_r % NLOC) // P
    rloc_e = g_r % P

    key = core_e * BLOCKS + bpos_e
    order_e = np.argsort(key, kind="stable")
    key_s = key[order_e]
    cnt = np.bincount(key, minlength=NB)
    starts = np.concatenate([[0], np.cumsum(cnt)[:-1]])
    rank = np.arange(N_EDGES, dtype=np.int64) - starts[key_s]
    bpos_s = bpos_e[order_e]
    core_s = core_e[order_e]
    assert (rank < u_profile[bpos_s] * P).all(), "chunk profile overflow"

    ci = cstart[bpos_s] + rank // P
    pp = rank % P
    flat = core_s * (P * nch) + pp * nch + ci

    cols_arr = np.zeros(NCORES * P * nch, np.int32)
    ew_arr = np.zeros(NCORES * P * nch, np.float32)
    rloc_arr = np.zeros(NCORES * P * nch, np.float32)
    cols_arr[flat] = g_c[order_e].astype(np.int32)
    ew_arr[flat] = np.asarray(edge_weight, np.float32)[order_e]
    rloc_arr[flat] = rloc_e[order_e].astype(np.float32)

    cols_arr = cols_arr.reshape(NCORES, P, nch)
    ew_arr = ew_arr.reshape(NCORES, P, nch)
    rloc_arr = rloc_arr.reshape(NCORES, P, nch)

    return perm, cols_arr, ew_arr, rloc_arr, tuple(u_profile.tolist()), cstart


def _build_program(u_profile, cstart):
    import concourse.bacc as bacc
    import concourse.bass as bass
    import concourse.mybir as mybir
    import concourse.tile as tile

    f32 = mybir.dt.float32
    f16 = mybir.dt.float16
    i32 = mybir.dt.int32
    AX = mybir.AxisListType.X
    AF = mybir.ActivationFunctionType
    OP = mybir.AluOpType
    nch = int(sum(u_profile))

    nc = bacc.Bacc("TRN2", target_bir_lowering=False, debug=False,
                   num_devices=NCORES)
    xT = nc.dram_tensor("xT", [NFEAT, NLOC], f16, kind="ExternalInput").ap()
    cols = nc.dram_tensor("cols", [P, nch], i32, kind="ExternalInput").ap()
    ewt = nc.dram_tensor("ewt", [P, nch], f16, kind="ExternalInput").ap()
    rlo = nc.dram_tensor("rlo", [P, nch], f16, kind="ExternalInput").ap()
    W1 = nc.dram_tensor("W1", [NFEAT, NHID], f16, kind="ExternalInput").ap()
    W2 = nc.dram_tensor("W2", [NHID, NHID], f16, kind="ExternalInput").ap()
    W3 = nc.dram_tensor("W3", [NHID, NCLASS], f16, kind="ExternalInput").ap()
    b1r = nc.dram_tensor("b1r", [P, NHID], f32, kind="ExternalInput").ap()
    b1c = nc.dram_tensor("b1c", [NHID, 1], f32, kind="ExternalInput").ap()
    b2r = nc.dram_tensor("b2r", [P, NHID], f32, kind="ExternalInput").ap()
    b3r = nc.dram_tensor("b3r", [P, NCLASS], f32, kind="ExternalInput").ap()
    iota = nc.dram_tensor("iota", [P, P], f16, kind="ExternalInput").ap()
    ident = nc.dram_tensor("ident", [P, P], f16, kind="ExternalInput").ap()
    out = nc.dram_tensor("out", [NLOC, NCLASS], f32, kind="ExternalOutput").ap()

    rg = [list(range(NCORES))]

    with tile.TileContext(nc) as tc:
        with (
            tc.tile_pool(name="consts", bufs=1) as cp,
            tc.tile_pool(name="dram", bufs=1, space="DRAM") as dp,
            tc.tile_pool(name="gather", bufs=16) as gp,
            tc.tile_pool(name="sel", bufs=8) as selp,
            tc.tile_pool(name="hblk", bufs=4) as hbp,
            tc.tile_pool(name="smax", bufs=4) as smp,
            tc.tile_pool(name="ps_spmm", bufs=2, space="PSUM") as ps_spmm,
            tc.tile_pool(name="ps_dense", bufs=2, space="PSUM") as ps_dense,
            tc.tile_pool(name="ps_b1", bufs=1, space="PSUM") as ps_b1,
            tc.tile_pool(name="ps_tr", bufs=2, space="PSUM") as ps_tr,
        ):
            # ---- constants into SBUF ----
            cols_sb = cp.tile([P, nch], i32)
            ew_sb = cp.tile([P, nch], f16)
            rlo_sb = cp.tile([P, nch], f16)
            iota_sb = cp.tile([P, P], f16)
            ident_sb = cp.tile([P, P], f16)
            b1c_sb = cp.tile([NHID, 1], f32)
            b2_sb = cp.tile([P, NHID], f32)
            b3_sb = cp.tile([P, NCLASS], f32)
            W2_sb = cp.tile([NHID, NHID], f16)
            W3_sb = cp.tile([NHID, NCLASS], f16)
            nc.sync.dma_start(out=cols_sb[:], in_=cols[:])
            nc.sync.dma_start(out=ew_sb[:], in_=ewt[:])
            nc.sync.dma_start(out=rlo_sb[:], in_=rlo[:])
            nc.sync.dma_start(out=iota_sb[:], in_=iota[:])
            nc.sync.dma_start(out=ident_sb[:], in_=ident[:])
            nc.sync.dma_start(out=b1c_sb[:], in_=b1c[:])
            nc.sync.dma_start(out=b2_sb[:], in_=b2r[:])
            nc.sync.dma_start(out=b3_sb[:], in_=b3r[:])
            nc.sync.dma_start(out=W2_sb[:], in_=W2[:])
            nc.sync.dma_start(out=W3_sb[:], in_=W3[:])
            W1_sb = []
            for kc in range(4):
                w = cp.tile([P, NHID], f16, name=f"W1_sb_{kc}")
                nc.sync.dma_start(out=w[:], in_=W1[kc * P:(kc + 1) * P, :])
                W1_sb.append(w)

            hT1_sb = cp.tile([NHID, NLOC], f16)
            hT2_sb = cp.tile([NHID, NLOC], f16)
            sup_sb = cp.tile([P, BLOCKS * NHID], f16)
            out_sb = cp.tile([P, BLOCKS * NCLASS], f32)

            # ---- internal DRAM ----
            sup1_l = dp.tile([NLOC, NHID], f16)
            sup2_l = dp.tile([NLOC, NHID], f16)
            sup3_l = dp.tile([NLOC, NCLASS], f16)
            T1 = dp.tile([NTOT, NHID], f16, addr_space="Shared")
            T2 = dp.tile([NTOT, NHID], f16, addr_space="Shared")
            T3 = dp.tile([NTOT, NCLASS], f16, addr_space="Shared")

            LO = SPLIT * P          # rows in AG half A (per core)
            HI = NLOC - LO
            TLO = NCORES * LO       # table rows in half A

            def store_and_ag(sup_l, T, fw):
                """Write sup_sb -> sup_l and AllGather into T, in two halves
                so the first collective overlaps the SPMM gather tail."""
                for (r0, r1, s0, s1) in ((0, LO, 0, SPLIT),
                                         (LO, NLOC, SPLIT, BLOCKS)):
                    nc.sync.dma_start(
                        out=sup_l[r0:r1].rearrange("(s p) f -> p s f", p=P),
                        in_=sup_sb[:, s0 * NHID:s1 * NHID].rearrange(
                            "p (s f) -> p s f", f=NHID)[:, :, :fw])
                nc.gpsimd.collective_compute(
                    "AllGather", OP.bypass, replica_groups=rg,
                    ins=[sup_l.opt()], outs=[T.opt()])

            # warm up the collective path while x loads
            warm_i = dp.tile([8, 8], f16)
            warm_o = dp.tile([64, 8], f16)
            wsb = cp.tile([8, 8], f16)
            nc.vector.memset(wsb[:], 0.0)
            nc.sync.dma_start(out=warm_i[:], in_=wsb[:])
            nc.gpsimd.collective_compute(
                "AllGather", OP.bypass, replica_groups=rg,
                ins=[warm_i.opt()], outs=[warm_o.opt()])

            # ---- phase A: support1 = x @ W1 + b1 (x shipped pre-transposed) ----
            # W1 stationary: psT[f, n] = sum_k W1[k, f] * xT[k, n], then
            # per-block PE transpose back to row-major support.
            strips = []
            for kc in range(4):
                s = cp.tile([P, NLOC], f16, name=f"xs_{kc}")
                nc.sync.dma_start(out=s[:, :LO], in_=xT[kc * P:(kc + 1) * P, :LO])
                strips.append(s)
            for kc in range(4):
                nc.sync.dma_start(out=strips[kc][:, LO:],
                                  in_=xT[kc * P:(kc + 1) * P, LO:])
            supT_sb = cp.tile([NHID, NLOC], f16)
            for g0 in range(0, BLOCKS, 4):
                w = min(4, BLOCKS - g0) * P
                psT = ps_b1.tile([NHID, 512], f32, space="PSUM")
                for kc in range(4):
                    nc.tensor.matmul(
                        psT[:, :w], lhsT=W1_sb[kc][:],
                        rhs=strips[kc][:, g0 * P:g0 * P + w],
                        start=(kc == 0), stop=(kc == 3))
                nc.scalar.activation(supT_sb[:, g0 * P:g0 * P + w], psT[:, :w],
                                     AF.Identity, bias=b1c_sb[:, 0:1])
            for m in range(BLOCKS):
                pst1 = ps_tr.tile([P, NHID], f16, space="PSUM", name="ps_t1", bufs=1)
                nc.tensor.transpose(pst1[:], supT_sb[:, m * P:(m + 1) * P],
                                    ident_sb[0:NHID, 0:NHID])
                nc.scalar.copy(sup_sb[:, m * NHID:(m + 1) * NHID], pst1[:])
            store_and_ag(sup1_l, T1, NHID)

            def spmm(table, dout, post):
                """SPMM over the allgathered table; post(ps, b) consumes the
                accumulated [P, dout] PSUM tile of block b."""
                for b in range(BLOCKS):
                    u = u_profile[b]
                    ps = ps_spmm.tile([P, NHID], f32, space="PSUM", name="ps_sp")
                    for k in range(u):
                        c = int(cstart[b]) + k
                        g = gp.tile([P, NHID], f16, name="g_t")
                        nc.gpsimd.indirect_dma_start(
                            out=g[:, :dout], out_offset=None, in_=table[:, :],
                            in_offset=bass.IndirectOffsetOnAxis(
                                ap=cols_sb[:, c:c + 1], axis=0))
                        # selw = (iota == rloc) * ew   (one fused DVE op)
                        sel = selp.tile([P, P], f16, name="sel_t")
                        nc.vector.scalar_tensor_tensor(
                            out=sel[:], in0=iota_sb[:],
                            scalar=rlo_sb[:, c:c + 1],
                            in1=ew_sb[:, c:c + 1].to_broadcast([P, P]),
                            op0=OP.is_equal, op1=OP.mult)
                        nc.tensor.matmul(ps[:, :dout], lhsT=sel[:],
                                         rhs=g[:, :dout],
                                         start=(k == 0), stop=(k == u - 1))
                    post(ps, b)

            def mk_post_h(hT_dst, func):
                def post(ps, b):
                    h = hbp.tile([P, NHID], f16, name="h_t")
                    nc.scalar.activation(h[:], ps[:], func)
                    pst = ps_tr.tile([NHID, P], f16, space="PSUM", name="ps_t")
                    nc.tensor.transpose(pst[:], h[:], ident_sb[:])
                    nc.scalar.copy(hT_dst[:, b * P:(b + 1) * P], pst[:])
                return post

            # ---- L1 SPMM + relu -> hT1 ----
            spmm(T1, NHID, mk_post_h(hT1_sb, AF.Relu))

            # ---- L2 dense + AG ----
            for m in range(BLOCKS):
                psd = ps_dense.tile([P, NHID], f32, space="PSUM")
                nc.tensor.matmul(psd[:], lhsT=hT1_sb[:, m * P:(m + 1) * P],
                                 rhs=W2_sb[:], start=True, stop=True)
                nc.vector.tensor_tensor(
                    out=sup_sb[:, m * NHID:(m + 1) * NHID],
                    in0=psd[:], in1=b2_sb[:], op=OP.add)
            store_and_ag(sup2_l, T2, NHID)

            # ---- L2 SPMM (no relu) -> hT2 ----
            spmm(T2, NHID, mk_post_h(hT2_sb, AF.Copy))

            # ---- L3 dense + AG ----
            for m in range(BLOCKS):
                psd = ps_dense.tile([P, NHID], f32, space="PSUM")
                nc.tensor.matmul(psd[:, :NCLASS],
                                 lhsT=hT2_sb[:, m * P:(m + 1) * P],
                                 rhs=W3_sb[:], start=True, stop=True)
                nc.vector.tensor_tensor(
                    out=sup_sb[:, m * NHID:m * NHID + NCLASS],
                    in0=psd[:, :NCLASS], in1=b3_sb[:], op=OP.add)
            store_and_ag(sup3_l, T3, NCLASS)

            # ---- L3 SPMM + log_softmax ----
            def post3(ps, b):
                negmax = smp.tile([P, 1], f32, name="negmax")
                nc.vector.reduce_max(out=negmax[:], in_=ps[:, :NCLASS], axis=AX,
                                     negate=True)
                t = hbp.tile([P, NHID], f32, name="t_t")
                nc.scalar.activation(t[:, :NCLASS], ps[:, :NCLASS], AF.Identity,
                                     bias=negmax[:, 0:1])
                e = hbp.tile([P, NHID], f32, name="e_t")
                nc.scalar.activation(e[:, :NCLASS], t[:, :NCLASS], AF.Exp)
                ssum = smp.tile([P, 1], f32, name="ssum")
                nc.vector.reduce_sum(out=ssum[:], in_=e[:, :NCLASS], axis=AX)
                nlg = smp.tile([P, 1], f32, name="nlg")
                nc.scalar.activation(nlg[:], ssum[:], AF.Ln)
                nc.vector.tensor_scalar_mul(nlg[:], nlg[:], -1.0)
                nc.scalar.activation(
                    out_sb[:, b * NCLASS:(b + 1) * NCLASS],
                    t[:, :NCLASS], AF.Identity, bias=nlg[:, 0:1])

            spmm(T3, NCLASS, post3)
            nc.sync.dma_start(
                out=out[:].rearrange("(s p) f -> p s f", p=P),
                in_=out_sb[:].rearrange("p (s f) -> p s f", f=NCLASS))

    nc.compile()
    return nc


def kernel(x, edge_weight, W1, b1, W2, b2, W3, b3, row, col):
    from concourse import bass_utils

    x = np.asarray(x, np.float32)
    edge_weight = np.asarray(edge_weight, np.float32)
    W1 = np.asarray(W1, np.float32)
    b1 = np.asarray(b1, np.float32)
    W2 = np.asarray(W2, np.float32)
    b2 = np.asarray(b2, np.float32)
    W3 = np.asarray(W3, np.float32)
    b3 = np.asarray(b3, np.float32)
    row = np.asarray(row, np.int64)
    col = np.asarray(col, np.int64)

    perm, cols_arr, ew_arr, rloc_arr, u_profile, cstart = _partition_graph(
        row, col, edge_weight)

    if u_profile not in _cache:
        _cache[u_profile] = _build_program(u_profile, cstart)
    nc = _cache[u_profile]

    x_new = np.zeros((NTOT, NFEAT), np.float32)
    x_new[perm] = x
    iota = np.tile(np.arange(P, dtype=np.float16), (P, 1))
    ident = np.eye(P, dtype=np.float16)
    in_maps = []
    for c in range(NCORES):
        in_maps.append({
            "xT": np.ascontiguousarray(
                x_new[c * NLOC:(c + 1) * NLOC].T).astype(np.float16),
            "cols": np.ascontiguousarray(cols_arr[c]),
            "ewt": np.ascontiguousarray(ew_arr[c]).astype(np.float16),
            "rlo": np.ascontiguousarray(rloc_arr[c]).astype(np.float16),
            "W1": W1.astype(np.float16), "W2": W2.astype(np.float16),
            "W3": W3.astype(np.float16),
            "b1r": np.tile(b1, (P, 1)), "b2r": np.tile(b2, (P, 1)),
            "b3r": np.tile(b3, (P, 1)),
            "b1c": b1.reshape(NHID, 1).astype(np.float32),
            "iota": iota, "ident": ident,
        })

    res = bass_utils.run_bass_kernel_spmd(
        nc, in_maps, core_ids=list(range(NCORES)),
        trace=kernel.trace)
    kernel.last_result = res

    full = np.concatenate([res.results[c]["out"] for c in range(NCORES)], axis=0)
    return full[perm[:N_NODES]].astype(np.float32)


kernel.trace = False
kernel.last_result = None


# revision 18
# speedup vs baseline: 1.0188x; 1.0188x over previous
"""3-layer GCN on 8 Trainium2 NeuronCores (SPMD, Bass/Tile).

Strategy: shard destination nodes across cores via a balanced LPT partition
into 128-row blocks; replicate weights; per layer: local dense matmul (fp16)
-> AllGather of the fp16 support table into Shared DRAM -> SPMM as
per-128-edge-chunk [indirect-DMA row gather (fp16, 128B rows); one fused DVE
op building the weight-scaled one-hot selection matrix; PE matmul
accumulating the segment-sum in PSUM].  All per-core variation lives in the
data (index/weight arrays); the program is identical across cores.
"""
import heapq

import numpy as np

N_NODES = 50000
N_EDGES = 800000
NFEAT, NHID, NCLASS = 512, 64, 40
NCORES = 8
P = 128
BLOCKS = 49                 # blocks per core
NLOC = BLOCKS * P           # 6272 rows per core
NB = NCORES * BLOCKS        # 392 blocks total
NTOT = NCORES * NLOC        # 50176 padded nodes
SPLIT = 24                  # blocks in the first AllGather half

_cache = {}


def _partition_graph(row, col, edge_weight):
    """Host-side graph partitioning. Returns permutation, per-core packed
    edge arrays, and the uniform per-block chunk profile."""
    deg = np.bincount(row, minlength=N_NODES)
    order = np.argsort(-deg, kind="stable")

    # LPT: assign nodes (desc degree) to the min-edge-sum block with a free slot
    heap = [(0, b) for b in range(NB)]
    heapq.heapify(heap)
    counts = np.zeros(NB, np.int64)
    bsum = np.zeros(NB, np.int64)
    assign = np.empty(N_NODES, np.int64)
    within = np.empty(N_NODES, np.int64)
    for n in order:
        s, b = heapq.heappop(heap)
        d = int(deg[n])
        assign[n] = b
        within[n] = counts[b]
        counts[b] += 1
        bsum[b] += d
        if counts[b] < P:
            heapq.heappush(heap, (s + d, b))

    # blocks -> cores: snake deal by desc edge sum; within-core slot = round idx
    border = np.argsort(-bsum, kind="stable")
    core_of_block = np.empty(NB, np.int64)
    slot_of_block = np.empty(NB, np.int64)
    for i, b in enumerate(border):
        r, j = divmod(i, NCORES)
        core_of_block[b] = j if r % 2 == 0 else NCORES - 1 - j
        slot_of_block[b] = r

    # per-slot chunk profile (uniform across cores)
    slot_max = np.zeros(BLOCKS, np.int64)
    for b in range(NB):
        r = slot_of_block[b]
        slot_max[r] = max(slot_max[r], bsum[b])
    u_profile = np.maximum(1, np.ceil(slot_max / P).astype(np.int64))
    cstart = np.concatenate([[0], np.cumsum(u_profile)[:-1]]).astype(np.int64)
    nch = int(u_profile.sum())

    # node permutation old -> new
    perm = (core_of_block[assign] * NLOC + slot_of_block[assign] * P + within)

    g_r = perm[row]
    g_c = perm[col]
    core_e = g_r // NLOC
    bpos_e = (g_r % NLOC) // P
    rloc_e = g_r % P

    key = core_e * BLOCKS + bpos_e
    order_e = np.argsort(key, kind="stable")
    key_s = key[order_e]
    cnt = np.bincount(key, minlength=NB)
    starts = np.concatenate([[0], np.cumsum(cnt)[:-1]])
    rank = np.arange(N_EDGES, dtype=np.int64) - starts[key_s]
    bpos_s = bpos_e[order_e]
    core_s = core_e[order_e]
    assert (rank < u_profile[bpos_s] * P).all(), "chunk profile overflow"

    ci = cstart[bpos_s] + rank // P
    pp = rank % P
    flat = core_s * (P * nch) + pp * nch + ci

    cols_arr = np.zeros(NCORES * P * nch, np.int32)
    ew_arr = np.zeros(NCORES * P * nch, np.float32)
    rloc_arr = np.zeros(NCORES * P * nch, np.float32)
    cols_arr[flat] = g_c[order_e].astype(np.int32)
    ew_arr[flat] = np.asarray(edge_weight, np.float32)[order_e]
    rloc_arr[flat] = rloc_e[order_e].astype(np.float32)

    cols_arr = cols_arr.reshape(NCORES, P, nch)
    ew_arr = ew_arr.reshape(NCORES, P, nch)
    rloc_arr = rloc_arr.reshape(NCORES, P, nch)

    return perm, cols_arr, ew_arr, rloc_arr, tuple(u_profile.tolist()), cstart


def _build_program(u_profile, cstart):
    import concourse.bacc as bacc
    import concourse.bass as bass
    import concourse.mybir as mybir
    import concourse.tile as tile

    f32 = mybir.dt.float32
    f16 = mybir.dt.float16
    i32 = mybir.dt.int32
    AX = mybir.AxisListType.X
    AF = mybir.ActivationFunctionType
    OP = mybir.AluOpType
    nch = int(sum(u_profile))

    nc = bacc.Bacc("TRN2", target_bir_lowering=False, debug=False,
                   num_devices=NCORES)
    xT = nc.dram_tensor("xT", [NFEAT, NLOC], f16, kind="ExternalInput").ap()
    cols = nc.dram_tensor("cols", [P, nch], i32, kind="ExternalInput").ap()
    ewt = nc.dram_tensor("ewt", [P, nch], f16, kind="ExternalInput").ap()
    rlo = nc.dram_tensor("rlo", [P, nch], f16, kind="ExternalInput").ap()
    W1 = nc.dram_tensor("W1", [NFEAT, NHID], f16, kind="ExternalInput").ap()
    W2 = nc.dram_tensor("W2", [NHID, NHID], f16, kind="ExternalInput").ap()
    W3 = nc.dram_tensor("W3", [NHID, NCLASS], f16, kind="ExternalInput").ap()
    b1r = nc.dram_tensor("b1r", [P, NHID], f32, kind="ExternalInput").ap()
    b2r = nc.dram_tensor("b2r", [P, NHID], f32, kind="ExternalInput").ap()
    b3r = nc.dram_tensor("b3r", [P, NCLASS], f32, kind="ExternalInput").ap()
    iota = nc.dram_tensor("iota", [P, P], f16, kind="ExternalInput").ap()
    ident = nc.dram_tensor("ident", [P, P], f16, kind="ExternalInput").ap()
    out = nc.dram_tensor("out", [NLOC, NCLASS], f32, kind="ExternalOutput").ap()

    rg = [list(range(NCORES))]

    with tile.TileContext(nc) as tc:
        with (
            tc.tile_pool(name="consts", bufs=1) as cp,
            tc.tile_pool(name="dram", bufs=1, space="DRAM") as dp,
            tc.tile_pool(name="gather", bufs=16) as gp,
            tc.tile_pool(name="sel", bufs=8) as selp,
            tc.tile_pool(name="hblk", bufs=4) as hbp,
            tc.tile_pool(name="smax", bufs=4) as smp,
            tc.tile_pool(name="ps_spmm", bufs=4, space="PSUM") as ps_spmm,
            tc.tile_pool(name="ps_dense", bufs=2, space="PSUM") as ps_dense,
            tc.tile_pool(name="ps_tr", bufs=2, space="PSUM") as ps_tr,
        ):
            # ---- constants into SBUF ----
            cols_sb = cp.tile([P, nch], i32)
            ew_sb = cp.tile([P, nch], f16)
            rlo_sb = cp.tile([P, nch], f16)
            iota_sb = cp.tile([P, P], f16)
            ident_sb = cp.tile([P, P], f16)
            b1_sb = cp.tile([P, NHID], f32)
            b2_sb = cp.tile([P, NHID], f32)
            b3_sb = cp.tile([P, NCLASS], f32)
            W2_sb = cp.tile([NHID, NHID], f16)
            W3_sb = cp.tile([NHID, NCLASS], f16)
            nc.sync.dma_start(out=cols_sb[:], in_=cols[:])
            nc.sync.dma_start(out=ew_sb[:], in_=ewt[:])
            nc.sync.dma_start(out=rlo_sb[:], in_=rlo[:])
            nc.sync.dma_start(out=iota_sb[:], in_=iota[:])
            nc.sync.dma_start(out=ident_sb[:], in_=ident[:])
            nc.sync.dma_start(out=b1_sb[:], in_=b1r[:])
            nc.sync.dma_start(out=b2_sb[:], in_=b2r[:])
            nc.sync.dma_start(out=b3_sb[:], in_=b3r[:])
            nc.sync.dma_start(out=W2_sb[:], in_=W2[:])
            nc.sync.dma_start(out=W3_sb[:], in_=W3[:])
            W1_sb = []
            for kc in range(4):
                w = cp.tile([P, NHID], f16, name=f"W1_sb_{kc}")
                nc.sync.dma_start(out=w[:], in_=W1[kc * P:(kc + 1) * P, :])
                W1_sb.append(w)

            hT1_sb = cp.tile([NHID, NLOC], f16)
            hT2_sb = cp.tile([NHID, NLOC], f16)
            sup_sb = cp.tile([P, BLOCKS * NHID], f16)
            out_sb = cp.tile([P, BLOCKS * NCLASS], f32)

            # ---- internal DRAM ----
            sup1_l = dp.tile([NLOC, NHID], f16)
            sup2_l = dp.tile([NLOC, NHID], f16)
            sup3_l = dp.tile([NLOC, NCLASS], f16)
            T1 = dp.tile([NTOT, NHID], f16, addr_space="Shared")
            T2 = dp.tile([NTOT, NHID], f16, addr_space="Shared")
            T3 = dp.tile([NTOT, NCLASS], f16, addr_space="Shared")

            LO = SPLIT * P          # rows in AG half A (per core)
            HI = NLOC - LO
            TLO = NCORES * LO       # table rows in half A

            def store_and_ag(sup_l, T, fw):
                """Write sup_sb -> sup_l and AllGather into T, in two halves
                so the first collective overlaps the SPMM gather tail."""
                for (r0, r1, s0, s1) in ((0, LO, 0, SPLIT),
                                         (LO, NLOC, SPLIT, BLOCKS)):
                    nc.sync.dma_start(
                        out=sup_l[r0:r1].rearrange("(s p) f -> p s f", p=P),
                        in_=sup_sb[:, s0 * NHID:s1 * NHID].rearrange(
                            "p (s f) -> p s f", f=NHID)[:, :, :fw])
                nc.gpsimd.collective_compute(
                    "AllGather", OP.bypass, replica_groups=rg,
                    ins=[sup_l.opt()], outs=[T.opt()])

            # ---- phase A: support1 = x @ W1 + b1 (x shipped pre-transposed) ----
            strips = []
            for kc in range(4):
                s = cp.tile([P, NLOC], f16, name=f"xs_{kc}")
                nc.sync.dma_start(out=s[:, :LO], in_=xT[kc * P:(kc + 1) * P, :LO])
                strips.append(s)
            for kc in range(4):
                nc.sync.dma_start(out=strips[kc][:, LO:],
                                  in_=xT[kc * P:(kc + 1) * P, LO:])
            for m in range(BLOCKS):
                psd = ps_dense.tile([P, NHID], f32, space="PSUM")
                for kc in range(4):
                    nc.tensor.matmul(
                        psd[:], lhsT=strips[kc][:, m * P:(m + 1) * P],
                        rhs=W1_sb[kc][:], start=(kc == 0), stop=(kc == 3))
                nc.vector.tensor_tensor(
                    out=sup_sb[:, m * NHID:(m + 1) * NHID],
                    in0=psd[:], in1=b1_sb[:], op=OP.add)
            store_and_ag(sup1_l, T1, NHID)

            def spmm(table, dout, post):
                """SPMM over the allgathered table; post(ps, b) consumes the
                accumulated [P, dout] PSUM tile of block b."""
                for b in range(BLOCKS):
                    u = u_profile[b]
                    ps = ps_spmm.tile([P, NHID], f32, space="PSUM", name="ps_sp")
                    for k in range(u):
                        c = int(cstart[b]) + k
                        g = gp.tile([P, NHID], f16, name="g_t")
                        nc.gpsimd.indirect_dma_start(
                            out=g[:, :dout], out_offset=None, in_=table[:, :],
                            in_offset=bass.IndirectOffsetOnAxis(
                                ap=cols_sb[:, c:c + 1], axis=0))
                        # selw = (iota == rloc) * ew   (one fused DVE op)
                        sel = selp.tile([P, P], f16, name="sel_t")
                        nc.vector.scalar_tensor_tensor(
                            out=sel[:], in0=iota_sb[:],
                            scalar=rlo_sb[:, c:c + 1],
                            in1=ew_sb[:, c:c + 1].to_broadcast([P, P]),
                            op0=OP.is_equal, op1=OP.mult)
                        nc.tensor.matmul(ps[:, :dout], lhsT=sel[:],
                                         rhs=g[:, :dout],
                                         start=(k == 0), stop=(k == u - 1))
                    post(ps, b)

            def mk_post_h(hT_dst, func):
                def post(ps, b):
                    h = hbp.tile([P, NHID], f16, name="h_t")
                    nc.scalar.activation(h[:], ps[:], func)
                    pst = ps_tr.tile([NHID, P], f16, space="PSUM", name="ps_t")
                    nc.tensor.transpose(pst[:], h[:], ident_sb[:])
                    nc.scalar.copy(hT_dst[:, b * P:(b + 1) * P], pst[:])
                return post

            # ---- L1 SPMM + relu -> hT1 ----
            spmm(T1, NHID, mk_post_h(hT1_sb, AF.Relu))

            # ---- L2 dense + AG ----
            for m in range(BLOCKS):
                psd = ps_dense.tile([P, NHID], f32, space="PSUM")
                nc.tensor.matmul(psd[:], lhsT=hT1_sb[:, m * P:(m + 1) * P],
                                 rhs=W2_sb[:], start=True, stop=True)
                nc.vector.tensor_tensor(
                    out=sup_sb[:, m * NHID:(m + 1) * NHID],
                    in0=psd[:], in1=b2_sb[:], op=OP.add)
            store_and_ag(sup2_l, T2, NHID)

            # ---- L2 SPMM (no relu) -> hT2 ----
            spmm(T2, NHID, mk_post_h(hT2_sb, AF.Copy))

            # ---- L3 dense + AG ----
            for m in range(BLOCKS):
                psd = ps_dense.tile([P, NHID], f32, space="PSUM")
                nc.tensor.matmul(psd[:, :NCLASS],
                                 lhsT=hT2_sb[:, m * P:(m + 1) * P],
                                 rhs=W3_sb[:], start=True, stop=True)
                nc.vector.tensor_tensor(
                    out=sup_sb[:, m * NHID:m * NHID + NCLASS],
                    in0=psd[:, :NCLASS], in1=b3_sb[:], op=OP.add)
            store_and_ag(sup3_l, T3, NCLASS)

            # ---- L3 SPMM + log_softmax ----
            def post3(ps, b):
                negmax = smp.tile([P, 1], f32, name="negmax")
                nc.vector.reduce_max(out=negmax[:], in_=ps[:, :NCLASS], axis=AX,
                                     negate=True)
                t = hbp.tile([P, NHID], f32, name="t_t")
                nc.scalar.activation(t[:, :NCLASS], ps[:, :NCLASS], AF.Identity,
                                     bias=negmax[:, 0:1])
                e = hbp.tile([P, NHID], f32, name="e_t")
                nc.scalar.activation(e[:, :NCLASS], t[:, :NCLASS], AF.Exp)
                ssum = smp.tile([P, 1], f32, name="ssum")
                nc.vector.reduce_sum(out=ssum[:], in_=e[:, :NCLASS], axis=AX)
                nlg = smp.tile([P, 1], f32, name="nlg")
                nc.scalar.activation(nlg[:], ssum[:], AF.Ln)
                nc.vector.tensor_scalar_mul(nlg[:], nlg[:], -1.0)
                nc.scalar.activation(
                    out_sb[:, b * NCLASS:(b + 1) * NCLASS],
                    t[:, :NCLASS], AF.Identity, bias=nlg[:, 0:1])

            spmm(T3, NCLASS, post3)
            nc.sync.dma_start(
                out=out[:].rearrange("(s p) f -> p s f", p=P),
                in_=out_sb[:].rearrange("p (s f) -> p s f", f=NCLASS))

    nc.compile()
    return nc


def kernel(x, edge_weight, W1, b1, W2, b2, W3, b3, row, col):
    from concourse import bass_utils

    x = np.asarray(x, np.float32)
    edge_weight = np.asarray(edge_weight, np.float32)
    W1 = np.asarray(W1, np.float32)
    b1 = np.asarray(b1, np.float32)
    W2 = np.asarray(W2, np.float32)
    b2 = np.asarray(b2, np.float32)
    W3 = np.asarray(W3, np.float32)
    b3 = np.asarray(b3, np.float32)
    row = np.asarray(row, np.int64)
    col = np.asarray(col, np.int64)

    perm, cols_arr, ew_arr, rloc_arr, u_profile, cstart = _partition_graph(
        row, col, edge_weight)

    if u_profile not in _cache:
        _cache[u_profile] = _build_program(u_profile, cstart)
    nc = _cache[u_profile]

    x_new = np.zeros((NTOT, NFEAT), np.float32)
    x_new[perm] = x
    iota = np.tile(np.arange(P, dtype=np.float16), (P, 1))
    ident = np.eye(P, dtype=np.float16)
    in_maps = []
    for c in range(NCORES):
        in_maps.append({
            "xT": np.ascontiguousarray(
                x_new[c * NLOC:(c + 1) * NLOC].T).astype(np.float16),
            "cols": np.ascontiguousarray(cols_arr[c]),
            "ewt": np.ascontiguousarray(ew_arr[c]).astype(np.float16),
            "rlo": np.ascontiguousarray(rloc_arr[c]).astype(np.float16),
            "W1": W1.astype(np.float16), "W2": W2.astype(np.float16),
            "W3": W3.astype(np.float16),
            "b1r": np.tile(b1, (P, 1)), "b2r": np.tile(b2, (P, 1)),
            "b3r": np.tile(b3, (P, 1)),
            "iota": iota, "ident": ident,
        })

    res = bass_utils.run_bass_kernel_spmd(
        nc, in_maps, core_ids=list(range(NCORES)),
        trace=kernel.trace)
    kernel.last_result = res

    full = np.concatenate([res.results[c]["out"] for c in range(NCORES)], axis=0)
    return full[perm[:N_NODES]].astype(np.float32)


kernel.trace = False
kernel.last_result = None
